# revision 33
# baseline (speedup 1.0000x reference)
"""Trainium2 Bass kernel for nn_LogicConv3d (differentiable-logic conv tree).

Problem (hardcoded): x [16,64,32,32] f32; idx_a/idx_b [64,900,64,3] i32;
w0..w6 [s,64,16] f32 (s = 64,32,16,8,4,2,1). Output [16,64,900,1] f32.

v3 design (fp16 product-form tree, 4-batch x 32-kernel sharding):
 - Sharding: core c handles batches [4*(c%4) .. +4) and kernels
   [32*(c//4) .. +32).  Wider rows (4 batches interleaved) halve the
   gather-descriptor count and per-op overheads vs batch-only sharding.
 - Algebra: every stored node value is an affine image X = (V - t)/s of the
   true node value V in [0,1].  Choosing the children's t as -C2/C3, -C1/C3
   makes each tree node an exact PRODUCT of its children's stored values:
   1 tensor_tensor(mult) + 1 tensor_scalar (affine fix) per node -- both
   have DVE fast modes in fp16, unlike scalar_tensor_tensor (none).
   L0 pre-shifts the raw leaves (2 extra ts).  All constants fold into the
   scalars; rel err ~7e-4 vs the 2e-2 gate (validated in emulation and on HW).
 - Gather: the x-slice is laid out in HBM as 9 shifted copies
   ([576, 4096] fp16 rows: (dh*3+dw)*64+c = channel-c image shifted by
   (dh,dw), 4 batches interleaved) so leaf crops are aligned rows fetched
   by batched dma_gather (int16 row ids, 256 rows / instruction).
 - Lanes: (group g = lane>>5, kernel k = lane&31); tree level l<=4 keeps
   groups independent; L5/L6 cross groups via SBUF->SBUF DMA realigns.
 - Compute is "wide" (3840 = 30h x 32w x 4b free elems, garbage at w=30,31)
   so all operands stay packed (DVE 4x/2x eligible); the final activation
   compacts to 900 windows x 4 batches in fp32.
"""
import numpy as np

B, C, H, W = 16, 64, 32, 32
K = 64
DEPTH = 6
PW = 30
P = PW * PW        # 900
NCORES = 8
B4 = 4             # batches per core
KPC = 32           # kernels per core
GRP = 4            # node groups in the lane dim
FW = 30 * 32 * B4  # 3840 wide free elems per lane
RSTEP = H * W * B4 # 4096: gather row stride (elements)
NSH = 9
ROWS = NSH * C     # 576 gather source rows
NT0 = 16           # L0 tiles
# gather entries: lists of side-indices (side s = 2t + (0:A,1:B)); a short
# first/last entry shrinks the pipeline ramp/tail, GB=4 in steady state
# keeps the GpSimd dispatch cost (~5us/instr) under the transfer time.
_GENT = [[0], [1]] + [[i, i + 1] for i in range(2, 32, 2)]
_ENT_OF_SIDE = {}
_ENT_COL = []      # cumulative int16-table column offset per entry
for _e, _sides in enumerate(_GENT):
    _ENT_COL.append(sum(8 * len(g) for g in _GENT[:_e]))
    for _j, _s in enumerate(_sides):
        _ENT_OF_SIDE[_s] = (_e, _j)
GCOLS = sum(8 * len(g) for g in _GENT)   # 256

GATE_M = np.array([
    [0, 0, 0, 0], [0, 0, 0, 1], [0, 1, 0, -1], [0, 1, 0, 0],
    [0, 0, 1, -1], [0, 0, 1, 0], [0, 1, 1, -2], [0, 1, 1, -1],
    [1, -1, -1, 1], [1, -1, -1, 2], [1, 0, -1, 0], [1, 0, -1, 1],
    [1, -1, 0, 0], [1, -1, 0, 1], [1, 0, 0, -1], [1, 0, 0, 0],
], dtype=np.float64)


def _softmax64(w):
    w = np.asarray(w, np.float64)
    e = np.exp(w - w.max(-1, keepdims=True))
    return e / e.sum(-1, keepdims=True)


# ---------------------------------------------------------------------------
# static op schedule with greedy engine assignment
# ---------------------------------------------------------------------------
def _schedule():
    """Software-pipelined op list.  Engines: 'v' DVE, 'a' Act; Pool only
    dispatches gathers (compute on Pool head-blocks the gather queue).
    Each L0 tile's combine ops (tt0/fix) are emitted one tile AFTER its
    side-ts ops, and merge ops one slot after the fix that enables them,
    so no op waits at an engine's queue head for a result another engine
    produced a moment ago.  The post-loop drain (final merge chain) is
    forced onto DVE -- it is latency-critical, and DVE has the cheapest
    per-op cost."""
    ops = []
    col = [0]
    loads = {'v': 0.0, 'a': 0.0}
    TSW = {'v': 1.38, 'a': 3.38}   # whole-tile tensor_scalar, measured us
    TTW = {'v': 2.15}              # whole-tile tensor_tensor
    ngather = [0]

    def pick(cost):
        e = min(cost, key=lambda k: loads[k] + cost[k])
        loads[e] += cost[e]
        return e

    def need_gather(upto):
        while ngather[0] <= min(upto, len(_GENT) - 1):
            ops.append({'kind': 'gather', 'g': ngather[0]})
            ngather[0] += 1

    def alloc_col():
        c = col[0]
        col[0] += 2
        return c

    fixed = set()
    mergeq = []

    def note_fixed(l, key, drain=False):
        fixed.add((l, key))
        if l < 4 and (l, key ^ 1) in fixed:
            mergeq.append((l + 1, key // 2))

    def emit_merge(drain, pool=False):
        l, key = mergeq.pop(0)
        c = alloc_col()
        e_tt = 'v' if drain else ('p' if pool else pick(TTW))
        e_ts = 'v' if drain else ('p' if pool else pick(TSW))
        ops.append({'kind': 'tt', 'l': l, 'key': key, 'eng': e_tt})
        ops.append({'kind': 'fix', 'l': l, 'key': key, 'col': c,
                    'eng': e_ts})
        note_fixed(l, key)

    pending = []
    for t in range(NT0):
        need_gather(_ENT_OF_SIDE[2 * t + 1][0] + 1)
        tail = t == NT0 - 1
        ops.append({'kind': 'ts_side', 'side': 0, 't': t,
                    'col': alloc_col(), 'eng': 'v' if tail else pick(TSW)})
        ops.append({'kind': 'ts_side', 'side': 1, 't': t,
                    'col': alloc_col(), 'eng': 'a' if tail else pick(TSW)})
        ops.extend(pending)
        if pending:
            note_fixed(0, t - 1)
        pending = [{'kind': 'tt0', 't': t,
                    'eng': 'v' if tail else pick(TTW)},
                   {'kind': 'fix', 'l': 0, 'key': t, 'col': alloc_col(),
                    'eng': 'v' if tail else pick(TSW)}]
        if mergeq:
            emit_merge(False)
        if mergeq:
            emit_merge(False)
    ops.extend(pending)
    note_fixed(0, NT0 - 1)
    while mergeq:
        emit_merge(True)
    # L5: node n merges groups (2n, 2n+1) of T4; realign to base-0 first.
    for n in (0, 1):
        c = alloc_col()
        ops.append({'kind': 'l5re', 'n': n})
        ops.append({'kind': 'tt5', 'n': n, 'eng': 'v'})
        ops.append({'kind': 'fix5', 'n': n, 'col': c, 'eng': 'v'})
    c = alloc_col()
    ops.append({'kind': 'tt6', 'eng': 'v'})
    for h in (0, 1):
        ops.append({'kind': 'fin', 'col': c, 'h': h})
    return ops, col[0]


_OPS, _NCOL = _schedule()

_LANES = np.arange(128)
_G = _LANES >> 5          # group 0..3
_KL = _LANES & 31         # kernel-in-core 0..31


def _node_of(l, key):
    """Tree-node index per lane for a level-l tile (l <= 4)."""
    if l == 0:
        return key + NT0 * _G
    return (_G << (4 - l)) + key


# ---------------------------------------------------------------------------
# host tables
# ---------------------------------------------------------------------------
def _build_tables(ws):
    """Per-(node,kern) scalars over the FULL K=64, f64."""
    cs = [np.einsum('skg,gj->skj', _softmax64(w), GATE_M) for w in ws]
    s_req = [np.ones((2 ** (DEPTH - l), K)) for l in range(DEPTH + 1)]
    t_req = [np.zeros((2 ** (DEPTH - l), K)) for l in range(DEPTH + 1)]
    for l in range(DEPTH, 0, -1):
        c = cs[l]
        c1, c2, c3 = c[..., 1], c[..., 2], c[..., 3]
        tA, tB = -c2 / c3, -c1 / c3
        t_req[l - 1][0::2], t_req[l - 1][1::2] = tA, tB
        s_req[l - 1][0::2] = (1 + np.abs(tA)) / 2
        s_req[l - 1][1::2] = (1 + np.abs(tB)) / 2
    c = cs[0]
    c1, c2, c3 = c[..., 1], c[..., 2], c[..., 3]
    ta, tb = -c2 / c3, -c1 / c3
    sa, sb = (1 + np.abs(ta)) / 2, (1 + np.abs(tb)) / 2
    l0ab = (1 / sa, -ta / sa, 1 / sb, -tb / sb)
    p2 = []
    for l in range(DEPTH + 1):
        c = cs[l]
        c0, c1, c2, c3 = c[..., 0], c[..., 1], c[..., 2], c[..., 3]
        if l == 0:
            tAc, tBc, sA, sB = ta, tb, sa, sb
        else:
            tAc, tBc = t_req[l - 1][0::2], t_req[l - 1][1::2]
            sA, sB = s_req[l - 1][0::2], s_req[l - 1][1::2]
        D0 = c0 + c1 * tAc + c2 * tBc + c3 * tAc * tBc
        p2.append((c3 * sA * sB / s_req[l], (D0 - t_req[l]) / s_req[l]))
    return l0ab, p2


def _coef_table(ws, kg):
    """[128, _NCOL] f32 for kernel-group kg (kernels 32kg..32kg+31)."""
    l0ab, p2 = _build_tables(ws)
    qa1, qa2, qb1, qb2 = l0ab
    kern = KPC * kg + _KL
    coef = np.zeros((128, _NCOL), dtype=np.float64)
    for op in _OPS:
        k = op['kind']
        if k == 'ts_side':
            s = _node_of(0, op['t'])
            q1, q2 = (qa1, qa2) if op['side'] == 0 else (qb1, qb2)
            coef[:, op['col']] = q1[s, kern]
            coef[:, op['col'] + 1] = q2[s, kern]
        elif k == 'fix':
            n = _node_of(op['l'], op['key'])
            al, be = p2[op['l']]
            coef[:, op['col']] = al[n, kern]
            coef[:, op['col'] + 1] = be[n, kern]
        elif k == 'fix5':
            al, be = p2[5]
            coef[0:32, op['col']] = al[op['n'], kern[0:32]]
            coef[0:32, op['col'] + 1] = be[op['n'], kern[0:32]]
        elif k == 'fin':
            al, be = p2[6]
            coef[0:32, op['col']] = al[0, kern[0:32]]
            coef[0:32, op['col'] + 1] = be[0, kern[0:32]]
    return coef.astype(np.float32)


def _gidx_table(idx_a, idx_b, kg):
    """int16 gather-row indices [128, GCOLS] for kernel-group kg.
    Entry e fetches sides _GENT[e]; row i = j*128 + p lands at
    table[i%16, _ENT_COL[e] + i//16]."""
    gidx = np.zeros((128, GCOLS), dtype=np.int64)
    kern = KPC * kg + _KL
    for e, sides in enumerate(_GENT):
        for j, sd in enumerate(sides):
            t, side = sd // 2, sd % 2
            idx = idx_a if side == 0 else idx_b
            s = _node_of(0, t)
            ha = idx[kern, 0, s, 0].astype(np.int64)
            wa = idx[kern, 0, s, 1].astype(np.int64)
            ca = idx[kern, 0, s, 2].astype(np.int64)
            val = (ha * 3 + wa) * C + ca
            i = j * 128 + _LANES
            gidx[i % 16, _ENT_COL[e] + i // 16] = val
    assert gidx.max() < ROWS
    return gidx.astype(np.int16)


def _xsh_core(x, bg):
    """[ROWS, RSTEP] fp16 for batch-group bg: row d*64+c = channel-c image
    shifted by d=(dh*3+dw), batches interleaved innermost."""
    xs = x[B4 * bg:B4 * bg + B4].transpose(1, 2, 3, 0)  # [C,H,W,B4]
    flat = np.zeros(C * H * W * B4 + RSTEP, dtype=np.float32)
    flat[:C * H * W * B4] = xs.reshape(-1)
    xsh = np.empty((ROWS, RSTEP), dtype=np.float16)
    for dh in range(3):
        for dw in range(3):
            d = dh * 3 + dw
            off = (dh * W + dw) * B4
            for c in range(C):
                base = c * (H * W * B4) + off
                xsh[d * C + c] = flat[base:base + RSTEP]
    return xsh


# ---------------------------------------------------------------------------
# numpy emulator of the exact device schedule (validation aid)
# ---------------------------------------------------------------------------
def _emulate_core(xsh, gidx, coef):
    def f16(v):
        return v.astype(np.float16).astype(np.float32)
    F2 = FW // 2
    xr = xsh.astype(np.float32)
    ab = {}
    tiles = {}
    tmp = {}
    x5 = {0: np.zeros((32, FW), np.float32), 1: np.zeros((32, FW), np.float32)}
    w5 = {}
    w6 = np.zeros((32, FW), np.float32)
    out = np.zeros((KPC, P * B4), dtype=np.float32)

    def hs(h):
        return slice(F2 * h, F2 * (h + 1))
    for op in _OPS:
        k = op['kind']
        if k == 'gather':
            e = op['g']
            nc = 8 * len(_GENT[e])
            cols = gidx[:, _ENT_COL[e]:_ENT_COL[e] + nc]
            flat = cols[:16, :].T.reshape(-1)
            dst = np.empty((128, len(_GENT[e]), FW), np.float32)
            for i, idx in enumerate(flat):
                dst[i % 128, i // 128] = xr[idx, :FW]
            ab[e] = dst
        elif k == 'ts_side':
            t, side = op['t'], op['side']
            e, j = _ENT_OF_SIDE[2 * t + side]
            a = ab[e][:, j, :]
            c = op['col']
            tmp[(t, side)] = f16(f16(a) * coef[:, c, None]
                                 + coef[:, c + 1, None])
        elif k == 'tt0':
            t = op['t']
            tmp[(t, 0)] = f16(tmp[(t, 0)] * tmp[(t, 1)])
        elif k == 'fix' and op['l'] == 0:
            c = op['col']
            tiles[(0, op['key'])] = f16(
                tmp[(op['key'], 0)] * coef[:, c, None] + coef[:, c + 1, None])
        elif k == 'tt':
            l, key = op['l'], op['key']
            tmp[('w', l, key)] = f16(tiles[(l - 1, 2 * key)] *
                                     tiles[(l - 1, 2 * key + 1)])
        elif k == 'fix':
            l, key, c = op['l'], op['key'], op['col']
            tiles[(l, key)] = f16(
                tmp[('w', l, key)] * coef[:, c, None] + coef[:, c + 1, None])
        elif k == 'l5re':
            pass
        elif k == 'tt5':
            n = op['n']
            T4 = tiles[(4, 0)]
            w5[n] = f16(T4[64 * n:64 * n + 32] * T4[64 * n + 32:64 * n + 64])
        elif k == 'fix5':
            n, c = op['n'], op['col']
            x5[n] = f16(
                w5[n] * coef[0:32, c, None] + coef[0:32, c + 1, None])
        elif k == 'tt6':
            w6 = f16(x5[0] * x5[1])
        elif k == 'fin':
            c, h = op['col'], op['h']
            o = (w6[:, hs(h)] * coef[0:32, c, None] + coef[0:32, c + 1, None])
            out[:, P * B4 // 2 * h:P * B4 // 2 * (h + 1)] = \
                o.reshape(KPC, 15, 32, B4)[:, :, :PW, :].reshape(KPC, -1)
    return out


# ---------------------------------------------------------------------------
# Bass program
# ---------------------------------------------------------------------------
_BASS_CACHE = {}


def _build_bass(debug=False):
    ck = ('nc', debug)
    if ck in _BASS_CACHE:
        return _BASS_CACHE[ck]
    import concourse.bass as bass  # noqa: F401
    import concourse.mybir as mybir
    import concourse.tile as tile
    import concourse.bacc as bacc

    f32 = mybir.dt.float32
    f16 = mybir.dt.float16
    i16 = mybir.dt.int16
    AL = mybir.AluOpType
    ACTF = mybir.ActivationFunctionType

    nc = bacc.Bacc("TRN2", target_bir_lowering=False, debug=debug,
                   num_devices=NCORES, num_swdge_queues=2)
    xsh_d = nc.dram_tensor("xsh", [ROWS, RSTEP], f16, kind="ExternalInput").ap()
    gidx_d = nc.dram_tensor("gidx", [128, GCOLS], i16,
                            kind="ExternalInput").ap()
    coef_d = nc.dram_tensor("coef", [128, _NCOL], f32,
                            kind="ExternalInput").ap()
    out_d = nc.dram_tensor("out", [KPC, P * B4], f32,
                           kind="ExternalOutput").ap()

    with tile.TileContext(nc) as tc:
        with (
            tc.tile_pool(name="const", bufs=1) as pc,
            tc.tile_pool(name="gath", bufs=2) as pg,
            tc.tile_pool(name="tmp", bufs=3) as ptmp,
            tc.tile_pool(name="t0", bufs=2) as pt0,
            tc.tile_pool(name="lvl", bufs=2) as plv,
            tc.tile_pool(name="fin", bufs=1) as pfin,
            tc.tile_pool(name="outp", bufs=1) as pout,
        ):
            gidx_t = pc.tile([128, GCOLS], i16, tag="gidx",
                             name="gidx_t")
            nc.sync.dma_start(gidx_t[:], gidx_d[:])
            coef_t = pc.tile([128, _NCOL], f32, tag="coef", name="coef_t")
            nc.sync.dma_start(coef_t[:], coef_d[:])
            warm_t = pc.tile([1, 8], f32, tag="warm", name="warm_t")
            nc.scalar.activation(warm_t[:], coef_t[0:1, 0:8],
                                 ACTF.Identity, bias=0.0, scale=1.0)

            eng = {'v': nc.vector, 'a': nc.scalar, 'p': nc.gpsimd}

            def ts(e, out_ap, in_ap, col, rows=slice(0, 128)):
                s1 = coef_t[rows, col:col + 1]
                s2 = coef_t[rows, col + 1:col + 2]
                if e == 'a':
                    nc.scalar.activation(out_ap, in_ap, ACTF.Identity,
                                         bias=s2, scale=s1)
                else:
                    eng[e].tensor_scalar(out=out_ap, in0=in_ap, scalar1=s1,
                                         scalar2=s2, op0=AL.mult, op1=AL.add)

            ab = {}
            tmp = {}
            tiles = {}
            x5 = {}
            ra5 = {}
            F2 = FW // 2
            xsh_view = xsh_d[:, 0:FW]

            def hsl(h):
                return slice(F2 * h, F2 * (h + 1))
            for op in _OPS:
                k = op['kind']
                if k == 'gather':
                    e = op['g']
                    ns = len(_GENT[e])
                    t_ab = pg.tile([128, 2 * FW], f16, tag="AB",
                                   name="ab")
                    ab[e] = t_ab
                    nc.gpsimd.dma_gather(
                        out_ap=t_ab[:, 0:ns * FW].rearrange(
                            "p (j e) -> p j e", j=ns, e=FW),
                        in_ap=xsh_view,
                        idxs_ap=gidx_t[:, _ENT_COL[e]:
                                       _ENT_COL[e] + 8 * ns],
                        num_idxs=128 * ns,
                        num_idxs_reg=128 * ns,
                        elem_size=FW,
                        elem_step=RSTEP,
                    )
                elif k == 'ts_side':
                    t, side = op['t'], op['side']
                    e, j = _ENT_OF_SIDE[2 * t + side]
                    src = ab[e][:, j * FW:(j + 1) * FW]
                    dst = ptmp.tile([128, FW], f16, tag="ab"[side],
                                    name="ab"[side])
                    tmp[(t, side)] = dst
                    ts(op['eng'], dst[:], src, op['col'])
                elif k == 'tt0':
                    t = op['t']
                    eng[op['eng']].tensor_tensor(
                        tmp[(t, 0)][:], tmp[(t, 1)][:], tmp[(t, 0)][:],
                        AL.mult)
                elif k == 'fix' and op['l'] == 0:
                    dst = pt0.tile([128, FW], f16, tag="T0",
                                   name=f"t0_{op['key']}")
                    tiles[(0, op['key'])] = dst
                    ts(op['eng'], dst[:], tmp[(op['key'], 0)][:], op['col'])
                elif k == 'tt':
                    l, key = op['l'], op['key']
                    tA = tiles[(l - 1, 2 * key)]
                    tB = tiles[(l - 1, 2 * key + 1)]
                    eng[op['eng']].tensor_tensor(
                        tA[:], tB[:], tA[:], AL.mult)
                elif k == 'fix':
                    l, key = op['l'], op['key']
                    pool = pfin if l == 4 else plv
                    tiles[(l, key)] = pool.tile(
                        [128, FW], f16, tag=f"T{l}", name=f"t{l}_{key}")
                    src = tiles[(l - 1, 2 * key)]
                    ts(op['eng'], tiles[(l, key)][:], src[:], op['col'])
                elif k == 'l5re':
                    # merges cross lane groups: copy T4 slabs to base 0
                    n = op['n']
                    T4 = tiles[(4, 0)]
                    ra = pfin.tile([32, FW], f16, tag=f"r{n}", name=f"r{n}")
                    nc.sync.dma_start(ra[:], T4[64 * n + 32:64 * n + 64, :])
                    if n == 0:
                        left = T4[0:32, :]
                    else:
                        rl = pfin.tile([32, FW], f16, tag="rl", name="rl")
                        nc.sync.dma_start(rl[:], T4[64:96, :])
                        left = rl[:]
                    ra5[n] = (left, ra)
                    if n == 1:
                        x5[1] = pfin.tile([32, FW], f16, tag="X51",
                                          name="x51")
                elif k == 'tt5':
                    n = op['n']
                    left, ra = ra5[n]
                    eng[op['eng']].tensor_tensor(
                        ra[:], left, ra[:], AL.mult)
                elif k == 'fix5':
                    n, c = op['n'], op['col']
                    if n == 0:
                        # write X5(0) straight over r0's buffer
                        x5[0] = ra5[0][1]
                        ts(op['eng'], x5[0][:], ra5[0][1][:],
                           c, rows=slice(0, 32))
                    else:
                        ts(op['eng'], x5[1][:], ra5[1][1][:],
                           c, rows=slice(0, 32))
                elif k == 'tt6':
                    eng[op['eng']].tensor_tensor(
                        x5[0][:], x5[1][:], x5[0][:], AL.mult)
                elif k == 'fin':
                    c, h = op['col'], op['h']
                    w6v = x5[0][:].rearrange(
                        "p (h w b) -> p h w b",
                        h=30, w=32, b=B4)[:, 15 * h:15 * h + 15, 0:PW, :]
                    half = P * B4 // 2
                    out_t = pout.tile([KPC, half], f32, tag=f"out{h}",
                                      name=f"outt{h}")
                    ov = out_t[:].rearrange(
                        "p (h w b) -> p h w b", h=15, w=PW, b=B4)
                    nc.scalar.activation(
                        ov, w6v, ACTF.Identity,
                        bias=coef_t[0:KPC, c + 1:c + 2],
                        scale=coef_t[0:KPC, c:c + 1])
                    nc.sync.dma_start(
                        out_d[:, half * h:half * (h + 1)], out_t[:])
    nc.compile()
    _BASS_CACHE[ck] = nc
    return nc


# ---------------------------------------------------------------------------
# entry points
# ---------------------------------------------------------------------------
def _prep_inputs(x, idx_a, idx_b, ws):
    x = np.ascontiguousarray(x, dtype=np.float32)
    coefs = [_coef_table(ws, kg) for kg in range(2)]
    gidxs = [_gidx_table(idx_a, idx_b, kg) for kg in range(2)]
    xshs = [_xsh_core(x, bg) for bg in range(4)]
    in_maps = []
    for core in range(NCORES):
        bg, kg = core % 4, core // 4
        in_maps.append({"xsh": xshs[bg], "gidx": gidxs[kg],
                        "coef": coefs[kg]})
    return in_maps


def _assemble(core_outs):
    full = np.zeros((B, K, P, 1), dtype=np.float32)
    for core, o in enumerate(core_outs):
        bg, kg = core % 4, core // 4
        o = np.asarray(o, np.float32).reshape(KPC, P, B4)
        full[B4 * bg:B4 * bg + B4, KPC * kg:KPC * kg + KPC, :, 0] = \
            o.transpose(2, 0, 1)
    return full


def kernel(x, idx_a, idx_b, w0, w1, w2, w3, w4, w5, w6):
    ws = [np.asarray(w, dtype=np.float32) for w in
          (w0, w1, w2, w3, w4, w5, w6)]
    x = np.asarray(x, dtype=np.float32)
    idx_a = np.asarray(idx_a, dtype=np.int32)
    idx_b = np.asarray(idx_b, dtype=np.int32)
    in_maps = _prep_inputs(x, idx_a, idx_b, ws)
    nc = _build_bass()
    from concourse.bass_utils import run_bass_kernel_spmd
    res = run_bass_kernel_spmd(nc, in_maps, core_ids=list(range(NCORES)))
    return _assemble([r["out"] for r in res.results])


def kernel_emulate(x, idx_a, idx_b, w0, w1, w2, w3, w4, w5, w6):
    """Pure-numpy emulation of the exact device schedule (debug aid)."""
    ws = [np.asarray(w, dtype=np.float32) for w in
          (w0, w1, w2, w3, w4, w5, w6)]
    in_maps = _prep_inputs(np.asarray(x, np.float32),
                           np.asarray(idx_a, np.int32),
                           np.asarray(idx_b, np.int32), ws)
    outs = [_emulate_core(m["xsh"], m["gidx"].astype(np.int64), m["coef"])
            for m in in_maps]
    return _assemble(outs)


# revision 35
# speedup vs baseline: 1.0127x; 1.0127x over previous
"""Trainium2 Bass kernel for nn_LogicConv3d (differentiable-logic conv tree).

Problem (hardcoded): x [16,64,32,32] f32; idx_a/idx_b [64,900,64,3] i32;
w0..w6 [s,64,16] f32 (s = 64,32,16,8,4,2,1). Output [16,64,900,1] f32.

v3 design (fp16 product-form tree, 4-batch x 32-kernel sharding):
 - Sharding: core c handles batches [4*(c%4) .. +4) and kernels
   [32*(c//4) .. +32).  Wider rows (4 batches interleaved) halve the
   gather-descriptor count and per-op overheads vs batch-only sharding.
 - Algebra: every stored node value is an affine image X = (V - t)/s of the
   true node value V in [0,1].  Choosing the children's t as -C2/C3, -C1/C3
   makes each tree node an exact PRODUCT of its children's stored values:
   1 tensor_tensor(mult) + 1 tensor_scalar (affine fix) per node -- both
   have DVE fast modes in fp16, unlike scalar_tensor_tensor (none).
   L0 pre-shifts the raw leaves (2 extra ts).  All constants fold into the
   scalars; rel err ~7e-4 vs the 2e-2 gate (validated in emulation and on HW).
 - Gather: the x-slice is laid out in HBM as 9 shifted copies
   ([576, 4096] fp16 rows: (dh*3+dw)*64+c = channel-c image shifted by
   (dh,dw), 4 batches interleaved) so leaf crops are aligned rows fetched
   by batched dma_gather (int16 row ids, 256 rows / instruction).
 - Lanes: (group g = lane>>5, kernel k = lane&31); tree level l<=4 keeps
   groups independent; L5/L6 cross groups via SBUF->SBUF DMA realigns.
 - Compute is "wide" (3840 = 30h x 32w x 4b free elems, garbage at w=30,31)
   so all operands stay packed (DVE 4x/2x eligible); the final activation
   compacts to 900 windows x 4 batches in fp32.
"""
import numpy as np

B, C, H, W = 16, 64, 32, 32
K = 64
DEPTH = 6
PW = 30
P = PW * PW        # 900
NCORES = 8
B4 = 4             # batches per core
KPC = 32           # kernels per core
GRP = 4            # node groups in the lane dim
FDAT = 30 * 30 * B4   # 3600 useful elems per lane (30h x 30w x 4b, packed)
FW = FDAT + 112       # padded to 3712 elems = 7424 B (29 x 256B, dma_gather)
RSTEP = FW            # gather rows are packed at the padded stride
NSH = 9
ROWS = NSH * C     # 576 gather source rows
NT0 = 16           # L0 tiles
# gather entries: lists of side-indices (side s = 2t + (0:A,1:B)); a short
# first/last entry shrinks the pipeline ramp/tail, GB=4 in steady state
# keeps the GpSimd dispatch cost (~5us/instr) under the transfer time.
_GENT = [[0], [1]] + [[i, i + 1] for i in range(2, 32, 2)]
_ENT_OF_SIDE = {}
_ENT_COL = []      # cumulative int16-table column offset per entry
for _e, _sides in enumerate(_GENT):
    _ENT_COL.append(sum(8 * len(g) for g in _GENT[:_e]))
    for _j, _s in enumerate(_sides):
        _ENT_OF_SIDE[_s] = (_e, _j)
GCOLS = sum(8 * len(g) for g in _GENT)   # 256

GATE_M = np.array([
    [0, 0, 0, 0], [0, 0, 0, 1], [0, 1, 0, -1], [0, 1, 0, 0],
    [0, 0, 1, -1], [0, 0, 1, 0], [0, 1, 1, -2], [0, 1, 1, -1],
    [1, -1, -1, 1], [1, -1, -1, 2], [1, 0, -1, 0], [1, 0, -1, 1],
    [1, -1, 0, 0], [1, -1, 0, 1], [1, 0, 0, -1], [1, 0, 0, 0],
], dtype=np.float64)


def _softmax64(w):
    w = np.asarray(w, np.float64)
    e = np.exp(w - w.max(-1, keepdims=True))
    return e / e.sum(-1, keepdims=True)


# ---------------------------------------------------------------------------
# static op schedule with greedy engine assignment
# ---------------------------------------------------------------------------
def _schedule():
    """Software-pipelined op list.  Engines: 'v' DVE, 'a' Act; Pool only
    dispatches gathers (compute on Pool head-blocks the gather queue).
    Each L0 tile's combine ops (tt0/fix) are emitted one tile AFTER its
    side-ts ops, and merge ops one slot after the fix that enables them,
    so no op waits at an engine's queue head for a result another engine
    produced a moment ago.  The post-loop drain (final merge chain) is
    forced onto DVE -- it is latency-critical, and DVE has the cheapest
    per-op cost."""
    ops = []
    col = [0]
    loads = {'v': 0.0, 'a': 0.0}
    TSW = {'v': 1.38, 'a': 3.38}   # whole-tile tensor_scalar, measured us
    TTW = {'v': 2.15}              # whole-tile tensor_tensor
    ngather = [0]

    def pick(cost):
        e = min(cost, key=lambda k: loads[k] + cost[k])
        loads[e] += cost[e]
        return e

    def need_gather(upto):
        while ngather[0] <= min(upto, len(_GENT) - 1):
            ops.append({'kind': 'gather', 'g': ngather[0]})
            ngather[0] += 1

    def alloc_col():
        c = col[0]
        col[0] += 2
        return c

    fixed = set()
    mergeq = []

    def note_fixed(l, key, drain=False):
        fixed.add((l, key))
        if l < 4 and (l, key ^ 1) in fixed:
            mergeq.append((l + 1, key // 2))

    def emit_merge(drain, pool=False):
        l, key = mergeq.pop(0)
        c = alloc_col()
        e_tt = 'v' if drain else ('p' if pool else pick(TTW))
        e_ts = 'v' if drain else ('p' if pool else pick(TSW))
        ops.append({'kind': 'tt', 'l': l, 'key': key, 'eng': e_tt})
        ops.append({'kind': 'fix', 'l': l, 'key': key, 'col': c,
                    'eng': e_ts})
        note_fixed(l, key)

    pending = []
    for t in range(NT0):
        need_gather(_ENT_OF_SIDE[2 * t + 1][0] + 1)
        tail = t == NT0 - 1
        ops.append({'kind': 'ts_side', 'side': 0, 't': t,
                    'col': alloc_col(), 'eng': 'v' if tail else pick(TSW)})
        ops.append({'kind': 'ts_side', 'side': 1, 't': t,
                    'col': alloc_col(), 'eng': 'a' if tail else pick(TSW)})
        ops.extend(pending)
        if pending:
            note_fixed(0, t - 1)
        pending = [{'kind': 'tt0', 't': t,
                    'eng': 'v' if tail else pick(TTW)},
                   {'kind': 'fix', 'l': 0, 'key': t, 'col': alloc_col(),
                    'eng': 'v' if tail else pick(TSW)}]
        if mergeq:
            emit_merge(False)
        if mergeq:
            emit_merge(False)
    ops.extend(pending)
    note_fixed(0, NT0 - 1)
    while mergeq:
        emit_merge(True)
    # L5: node n merges groups (2n, 2n+1) of T4; realign to base-0 first.
    for n in (0, 1):
        c = alloc_col()
        ops.append({'kind': 'l5re', 'n': n})
        ops.append({'kind': 'tt5', 'n': n, 'eng': 'v'})
        ops.append({'kind': 'fix5', 'n': n, 'col': c, 'eng': 'v'})
    c = alloc_col()
    ops.append({'kind': 'tt6', 'eng': 'v'})
    for h in (0, 1):
        ops.append({'kind': 'fin', 'col': c, 'h': h})
    return ops, col[0]


_OPS, _NCOL = _schedule()

_LANES = np.arange(128)
_G = _LANES >> 5          # group 0..3
_KL = _LANES & 31         # kernel-in-core 0..31


def _node_of(l, key):
    """Tree-node index per lane for a level-l tile (l <= 4)."""
    if l == 0:
        return key + NT0 * _G
    return (_G << (4 - l)) + key


# ---------------------------------------------------------------------------
# host tables
# ---------------------------------------------------------------------------
def _build_tables(ws):
    """Per-(node,kern) scalars over the FULL K=64, f64."""
    cs = [np.einsum('skg,gj->skj', _softmax64(w), GATE_M) for w in ws]
    s_req = [np.ones((2 ** (DEPTH - l), K)) for l in range(DEPTH + 1)]
    t_req = [np.zeros((2 ** (DEPTH - l), K)) for l in range(DEPTH + 1)]
    for l in range(DEPTH, 0, -1):
        c = cs[l]
        c1, c2, c3 = c[..., 1], c[..., 2], c[..., 3]
        tA, tB = -c2 / c3, -c1 / c3
        t_req[l - 1][0::2], t_req[l - 1][1::2] = tA, tB
        s_req[l - 1][0::2] = (1 + np.abs(tA)) / 2
        s_req[l - 1][1::2] = (1 + np.abs(tB)) / 2
    c = cs[0]
    c1, c2, c3 = c[..., 1], c[..., 2], c[..., 3]
    ta, tb = -c2 / c3, -c1 / c3
    sa, sb = (1 + np.abs(ta)) / 2, (1 + np.abs(tb)) / 2
    l0ab = (1 / sa, -ta / sa, 1 / sb, -tb / sb)
    p2 = []
    for l in range(DEPTH + 1):
        c = cs[l]
        c0, c1, c2, c3 = c[..., 0], c[..., 1], c[..., 2], c[..., 3]
        if l == 0:
            tAc, tBc, sA, sB = ta, tb, sa, sb
        else:
            tAc, tBc = t_req[l - 1][0::2], t_req[l - 1][1::2]
            sA, sB = s_req[l - 1][0::2], s_req[l - 1][1::2]
        D0 = c0 + c1 * tAc + c2 * tBc + c3 * tAc * tBc
        p2.append((c3 * sA * sB / s_req[l], (D0 - t_req[l]) / s_req[l]))
    return l0ab, p2


def _coef_table(ws, kg):
    """[128, _NCOL] f32 for kernel-group kg (kernels 32kg..32kg+31)."""
    l0ab, p2 = _build_tables(ws)
    qa1, qa2, qb1, qb2 = l0ab
    kern = KPC * kg + _KL
    coef = np.zeros((128, _NCOL), dtype=np.float64)
    for op in _OPS:
        k = op['kind']
        if k == 'ts_side':
            s = _node_of(0, op['t'])
            q1, q2 = (qa1, qa2) if op['side'] == 0 else (qb1, qb2)
            coef[:, op['col']] = q1[s, kern]
            coef[:, op['col'] + 1] = q2[s, kern]
        elif k == 'fix':
            n = _node_of(op['l'], op['key'])
            al, be = p2[op['l']]
            coef[:, op['col']] = al[n, kern]
            coef[:, op['col'] + 1] = be[n, kern]
        elif k == 'fix5':
            al, be = p2[5]
            coef[0:32, op['col']] = al[op['n'], kern[0:32]]
            coef[0:32, op['col'] + 1] = be[op['n'], kern[0:32]]
        elif k == 'fin':
            al, be = p2[6]
            coef[0:32, op['col']] = al[0, kern[0:32]]
            coef[0:32, op['col'] + 1] = be[0, kern[0:32]]
    return coef.astype(np.float32)


def _gidx_table(idx_a, idx_b, kg):
    """int16 gather-row indices [128, GCOLS] for kernel-group kg.
    Entry e fetches sides _GENT[e]; row i = j*128 + p lands at
    table[i%16, _ENT_COL[e] + i//16]."""
    gidx = np.zeros((128, GCOLS), dtype=np.int64)
    kern = KPC * kg + _KL
    for e, sides in enumerate(_GENT):
        for j, sd in enumerate(sides):
            t, side = sd // 2, sd % 2
            idx = idx_a if side == 0 else idx_b
            s = _node_of(0, t)
            ha = idx[kern, 0, s, 0].astype(np.int64)
            wa = idx[kern, 0, s, 1].astype(np.int64)
            ca = idx[kern, 0, s, 2].astype(np.int64)
            val = (ha * 3 + wa) * C + ca
            i = j * 128 + _LANES
            gidx[i % 16, _ENT_COL[e] + i // 16] = val
    assert gidx.max() < ROWS
    return gidx.astype(np.int16)


def _xsh_core(x, bg):
    """[ROWS, RSTEP] fp16 for batch-group bg: row d*64+c = the compact
    30x30x4b crop of channel c at shift d=(dh*3+dw), zero-padded to FW."""
    xs = x[B4 * bg:B4 * bg + B4].transpose(1, 2, 3, 0)  # [C,H,W,B4]
    xsh = np.zeros((ROWS, RSTEP), dtype=np.float16)
    for dh in range(3):
        for dw in range(3):
            d = dh * 3 + dw
            for c in range(C):
                xsh[d * C + c, :FDAT] = \
                    xs[c, dh:dh + 30, dw:dw + 30, :].reshape(-1)
    return xsh


# ---------------------------------------------------------------------------
# numpy emulator of the exact device schedule (validation aid)
# ---------------------------------------------------------------------------
def _emulate_core(xsh, gidx, coef):
    def f16(v):
        return v.astype(np.float16).astype(np.float32)
    F2 = FW // 2
    xr = xsh.astype(np.float32)
    ab = {}
    tiles = {}
    tmp = {}
    x5 = {0: np.zeros((32, FW), np.float32), 1: np.zeros((32, FW), np.float32)}
    w5 = {}
    w6 = np.zeros((32, FW), np.float32)
    out = np.zeros((KPC, P * B4), dtype=np.float32)

    def hs(h):
        return slice(F2 * h, F2 * (h + 1))
    for op in _OPS:
        k = op['kind']
        if k == 'gather':
            e = op['g']
            nc = 8 * len(_GENT[e])
            cols = gidx[:, _ENT_COL[e]:_ENT_COL[e] + nc]
            flat = cols[:16, :].T.reshape(-1)
            dst = np.empty((128, len(_GENT[e]), FW), np.float32)
            for i, idx in enumerate(flat):
                dst[i % 128, i // 128] = xr[idx, :FW]
            ab[e] = dst
        elif k == 'ts_side':
            t, side = op['t'], op['side']
            e, j = _ENT_OF_SIDE[2 * t + side]
            a = ab[e][:, j, :]
            c = op['col']
            tmp[(t, side)] = f16(f16(a) * coef[:, c, None]
                                 + coef[:, c + 1, None])
        elif k == 'tt0':
            t = op['t']
            tmp[(t, 0)] = f16(tmp[(t, 0)] * tmp[(t, 1)])
        elif k == 'fix' and op['l'] == 0:
            c = op['col']
            tiles[(0, op['key'])] = f16(
                tmp[(op['key'], 0)] * coef[:, c, None] + coef[:, c + 1, None])
        elif k == 'tt':
            l, key = op['l'], op['key']
            tmp[('w', l, key)] = f16(tiles[(l - 1, 2 * key)] *
                                     tiles[(l - 1, 2 * key + 1)])
        elif k == 'fix':
            l, key, c = op['l'], op['key'], op['col']
            tiles[(l, key)] = f16(
                tmp[('w', l, key)] * coef[:, c, None] + coef[:, c + 1, None])
        elif k == 'l5re':
            pass
        elif k == 'tt5':
            n = op['n']
            T4 = tiles[(4, 0)]
            w5[n] = f16(T4[64 * n:64 * n + 32] * T4[64 * n + 32:64 * n + 64])
        elif k == 'fix5':
            n, c = op['n'], op['col']
            x5[n] = f16(
                w5[n] * coef[0:32, c, None] + coef[0:32, c + 1, None])
        elif k == 'tt6':
            w6 = f16(x5[0] * x5[1])
        elif k == 'fin':
            c, h = op['col'], op['h']
            half = P * B4 // 2
            o = (w6[:, half * h:half * (h + 1)] * coef[0:32, c, None]
                 + coef[0:32, c + 1, None])
            out[:, half * h:half * (h + 1)] = o
    return out


# ---------------------------------------------------------------------------
# Bass program
# ---------------------------------------------------------------------------
_BASS_CACHE = {}


def _build_bass(debug=False):
    ck = ('nc', debug)
    if ck in _BASS_CACHE:
        return _BASS_CACHE[ck]
    import concourse.bass as bass  # noqa: F401
    import concourse.mybir as mybir
    import concourse.tile as tile
    import concourse.bacc as bacc

    f32 = mybir.dt.float32
    f16 = mybir.dt.float16
    i16 = mybir.dt.int16
    AL = mybir.AluOpType
    ACTF = mybir.ActivationFunctionType

    nc = bacc.Bacc("TRN2", target_bir_lowering=False, debug=debug,
                   num_devices=NCORES, num_swdge_queues=2)
    xsh_d = nc.dram_tensor("xsh", [ROWS, RSTEP], f16, kind="ExternalInput").ap()
    gidx_d = nc.dram_tensor("gidx", [128, GCOLS], i16,
                            kind="ExternalInput").ap()
    coef_d = nc.dram_tensor("coef", [128, _NCOL], f32,
                            kind="ExternalInput").ap()
    out_d = nc.dram_tensor("out", [KPC, P * B4], f32,
                           kind="ExternalOutput").ap()

    with tile.TileContext(nc) as tc:
        with (
            tc.tile_pool(name="const", bufs=1) as pc,
            tc.tile_pool(name="gath", bufs=2) as pg,
            tc.tile_pool(name="tmp", bufs=3) as ptmp,
            tc.tile_pool(name="t0", bufs=2) as pt0,
            tc.tile_pool(name="lvl", bufs=2) as plv,
            tc.tile_pool(name="fin", bufs=1) as pfin,
            tc.tile_pool(name="outp", bufs=1) as pout,
        ):
            gidx_t = pc.tile([128, GCOLS], i16, tag="gidx",
                             name="gidx_t")
            nc.sync.dma_start(gidx_t[:], gidx_d[:])
            coef_t = pc.tile([128, _NCOL], f32, tag="coef", name="coef_t")
            nc.sync.dma_start(coef_t[:], coef_d[:])
            warm_t = pc.tile([1, 8], f32, tag="warm", name="warm_t")
            nc.scalar.activation(warm_t[:], coef_t[0:1, 0:8],
                                 ACTF.Identity, bias=0.0, scale=1.0)

            eng = {'v': nc.vector, 'a': nc.scalar, 'p': nc.gpsimd}

            def ts(e, out_ap, in_ap, col, rows=slice(0, 128)):
                s1 = coef_t[rows, col:col + 1]
                s2 = coef_t[rows, col + 1:col + 2]
                if e == 'a':
                    nc.scalar.activation(out_ap, in_ap, ACTF.Identity,
                                         bias=s2, scale=s1)
                else:
                    eng[e].tensor_scalar(out=out_ap, in0=in_ap, scalar1=s1,
                                         scalar2=s2, op0=AL.mult, op1=AL.add)

            ab = {}
            tmp = {}
            tiles = {}
            x5 = {}
            ra5 = {}
            F2 = FW // 2
            xsh_view = xsh_d[:, 0:FW]

            def hsl(h):
                return slice(F2 * h, F2 * (h + 1))
            for op in _OPS:
                k = op['kind']
                if k == 'gather':
                    e = op['g']
                    ns = len(_GENT[e])
                    t_ab = pg.tile([128, 2 * FW], f16, tag="AB",
                                   name="ab")
                    ab[e] = t_ab
                    nc.gpsimd.dma_gather(
                        out_ap=t_ab[:, 0:ns * FW].rearrange(
                            "p (j e) -> p j e", j=ns, e=FW),
                        in_ap=xsh_view,
                        idxs_ap=gidx_t[:, _ENT_COL[e]:
                                       _ENT_COL[e] + 8 * ns],
                        num_idxs=128 * ns,
                        num_idxs_reg=128 * ns,
                        elem_size=FW,
                        elem_step=RSTEP,
                    )
                elif k == 'ts_side':
                    t, side = op['t'], op['side']
                    e, j = _ENT_OF_SIDE[2 * t + side]
                    src = ab[e][:, j * FW:(j + 1) * FW]
                    dst = ptmp.tile([128, FW], f16, tag="ab"[side],
                                    name="ab"[side])
                    tmp[(t, side)] = dst
                    ts(op['eng'], dst[:], src, op['col'])
                elif k == 'tt0':
                    t = op['t']
                    eng[op['eng']].tensor_tensor(
                        tmp[(t, 0)][:], tmp[(t, 1)][:], tmp[(t, 0)][:],
                        AL.mult)
                elif k == 'fix' and op['l'] == 0:
                    dst = pt0.tile([128, FW], f16, tag="T0",
                                   name=f"t0_{op['key']}")
                    tiles[(0, op['key'])] = dst
                    ts(op['eng'], dst[:], tmp[(op['key'], 0)][:], op['col'])
                elif k == 'tt':
                    l, key = op['l'], op['key']
                    tA = tiles[(l - 1, 2 * key)]
                    tB = tiles[(l - 1, 2 * key + 1)]
                    eng[op['eng']].tensor_tensor(
                        tA[:], tB[:], tA[:], AL.mult)
                elif k == 'fix':
                    l, key = op['l'], op['key']
                    pool = pfin if l == 4 else plv
                    tiles[(l, key)] = pool.tile(
                        [128, FW], f16, tag=f"T{l}", name=f"t{l}_{key}")
                    src = tiles[(l - 1, 2 * key)]
                    ts(op['eng'], tiles[(l, key)][:], src[:], op['col'])
                elif k == 'l5re':
                    # merges cross lane groups: copy T4 slabs to base 0
                    n = op['n']
                    T4 = tiles[(4, 0)]
                    ra = pfin.tile([32, FW], f16, tag=f"r{n}", name=f"r{n}")
                    nc.sync.dma_start(ra[:], T4[64 * n + 32:64 * n + 64, :])
                    if n == 0:
                        left = T4[0:32, :]
                    else:
                        rl = pfin.tile([32, FW], f16, tag="rl", name="rl")
                        nc.sync.dma_start(rl[:], T4[64:96, :])
                        left = rl[:]
                    ra5[n] = (left, ra)
                    if n == 1:
                        x5[1] = pfin.tile([32, FW], f16, tag="X51",
                                          name="x51")
                elif k == 'tt5':
                    n = op['n']
                    left, ra = ra5[n]
                    eng[op['eng']].tensor_tensor(
                        ra[:], left, ra[:], AL.mult)
                elif k == 'fix5':
                    n, c = op['n'], op['col']
                    if n == 0:
                        # write X5(0) straight over r0's buffer
                        x5[0] = ra5[0][1]
                        ts(op['eng'], x5[0][:], ra5[0][1][:],
                           c, rows=slice(0, 32))
                    else:
                        ts(op['eng'], x5[1][:], ra5[1][1][:],
                           c, rows=slice(0, 32))
                elif k == 'tt6':
                    eng[op['eng']].tensor_tensor(
                        x5[0][:], x5[1][:], x5[0][:], AL.mult)
                elif k == 'fin':
                    # compact layout: output halves are contiguous slices
                    c, h = op['col'], op['h']
                    half = P * B4 // 2
                    w6v = x5[0][:, half * h:half * (h + 1)]
                    out_t = pout.tile([KPC, half], f32, tag=f"out{h}",
                                      name=f"outt{h}")
                    nc.scalar.activation(
                        out_t[:], w6v, ACTF.Identity,
                        bias=coef_t[0:KPC, c + 1:c + 2],
                        scale=coef_t[0:KPC, c:c + 1])
                    nc.sync.dma_start(
                        out_d[:, half * h:half * (h + 1)], out_t[:])
    nc.compile()
    _BASS_CACHE[ck] = nc
    return nc


# ---------------------------------------------------------------------------
# entry points
# ---------------------------------------------------------------------------
def _prep_inputs(x, idx_a, idx_b, ws):
    x = np.ascontiguousarray(x, dtype=np.float32)
    coefs = [_coef_table(ws, kg) for kg in range(2)]
    gidxs = [_gidx_table(idx_a, idx_b, kg) for kg in range(2)]
    xshs = [_xsh_core(x, bg) for bg in range(4)]
    in_maps = []
    for core in range(NCORES):
        bg, kg = core % 4, core // 4
        in_maps.append({"xsh": xshs[bg], "gidx": gidxs[kg],
                        "coef": coefs[kg]})
    return in_maps


def _assemble(core_outs):
    full = np.zeros((B, K, P, 1), dtype=np.float32)
    for core, o in enumerate(core_outs):
        bg, kg = core % 4, core // 4
        o = np.asarray(o, np.float32).reshape(KPC, P, B4)
        full[B4 * bg:B4 * bg + B4, KPC * kg:KPC * kg + KPC, :, 0] = \
            o.transpose(2, 0, 1)
    return full


def kernel(x, idx_a, idx_b, w0, w1, w2, w3, w4, w5, w6):
    ws = [np.asarray(w, dtype=np.float32) for w in
          (w0, w1, w2, w3, w4, w5, w6)]
    x = np.asarray(x, dtype=np.float32)
    idx_a = np.asarray(idx_a, dtype=np.int32)
    idx_b = np.asarray(idx_b, dtype=np.int32)
    in_maps = _prep_inputs(x, idx_a, idx_b, ws)
    nc = _build_bass()
    from concourse.bass_utils import run_bass_kernel_spmd
    res = run_bass_kernel_spmd(nc, in_maps, core_ids=list(range(NCORES)))
    return _assemble([r["out"] for r in res.results])


def kernel_emulate(x, idx_a, idx_b, w0, w1, w2, w3, w4, w5, w6):
    """Pure-numpy emulation of the exact device schedule (debug aid)."""
    ws = [np.asarray(w, dtype=np.float32) for w in
          (w0, w1, w2, w3, w4, w5, w6)]
    in_maps = _prep_inputs(np.asarray(x, np.float32),
                           np.asarray(idx_a, np.int32),
                           np.asarray(idx_b, np.int32), ws)
    outs = [_emulate_core(m["xsh"], m["gidx"].astype(np.int64), m["coef"])
            for m in in_maps]
    return _assemble(outs)


# revision 36
# speedup vs baseline: 1.0133x; 1.0006x over previous
"""Trainium2 Bass kernel for nn_LogicConv3d (differentiable-logic conv tree).

Problem (hardcoded): x [16,64,32,32] f32; idx_a/idx_b [64,900,64,3] i32;
w0..w6 [s,64,16] f32 (s = 64,32,16,8,4,2,1). Output [16,64,900,1] f32.

Final design (fp16 product-form tree, 4-batch x 32-kernel sharding):
 - Sharding: core c handles batches [4*(c%4) .. +4) and kernels
   [32*(c//4) .. +32).  Wide rows (4 batches interleaved) halve the
   gather-descriptor count and per-op overheads vs batch-only sharding.
 - Algebra: every stored node value is an affine image X = (V - t)/s of the
   true node value V in [0,1].  Choosing the children's t as -C2/C3, -C1/C3
   makes each tree node an exact PRODUCT of its children's stored values:
   1 tensor_tensor(mult) + 1 tensor_scalar (affine fix) per node -- both
   have DVE fast modes in fp16, unlike scalar_tensor_tensor (none).
   L0 pre-shifts the raw leaves (2 extra ts).  All constants fold into the
   scalars; rel err ~7e-4 vs the 2e-2 gate (validated in emulation and on HW).
 - Gather: the x-slice is laid out in HBM as 9*64 compact crop rows
   ([576, 3712] fp16: row (dh*3+dw)*64+c = the 30x30x4b crop of channel c
   at shift (dh,dw), zero-padded 3600->3712 elems = 29x256B so batched
   dma_gather (int16 row ids, <=256 rows/instruction) stays legal).
 - Lanes: (group g = lane>>5, kernel k = lane&31); tree level l<=4 keeps
   groups independent; L5/L6 cross groups via SBUF->SBUF DMA realigns
   (multi-operand engine ops need equal partition bases).
 - Schedule: software-pipelined emission (each tile's combine ops trail its
   side-ts by one tile; merges trail the enabling fix by one slot) with
   greedy DVE/Act load balancing from measured per-op costs; Pool only
   dispatches gathers (compute on it head-blocks the SWDGE queue); the
   final merge cascade is forced onto DVE (latency-critical).
   All operands stay packed (DVE 4x/2x eligible); the final activation
   emits 900 windows x 4 batches in fp32 as contiguous halves.
"""
import numpy as np

B, C, H, W = 16, 64, 32, 32
K = 64
DEPTH = 6
PW = 30
P = PW * PW        # 900
NCORES = 8
B4 = 4             # batches per core
KPC = 32           # kernels per core
GRP = 4            # node groups in the lane dim
FDAT = 30 * 30 * B4   # 3600 useful elems per lane (30h x 30w x 4b, packed)
FW = FDAT + 112       # padded to 3712 elems = 7424 B (29 x 256B, dma_gather)
RSTEP = FW            # gather rows are packed at the padded stride
NSH = 9
ROWS = NSH * C     # 576 gather source rows
NT0 = 16           # L0 tiles
# gather entries: lists of side-indices (side s = 2t + (0:A,1:B)); a short
# first/last entry shrinks the pipeline ramp/tail, GB=4 in steady state
# keeps the GpSimd dispatch cost (~5us/instr) under the transfer time.
_GENT = [[0], [1]] + [[i, i + 1] for i in range(2, 32, 2)]
_ENT_OF_SIDE = {}
_ENT_COL = []      # cumulative int16-table column offset per entry
for _e, _sides in enumerate(_GENT):
    _ENT_COL.append(sum(8 * len(g) for g in _GENT[:_e]))
    for _j, _s in enumerate(_sides):
        _ENT_OF_SIDE[_s] = (_e, _j)
GCOLS = sum(8 * len(g) for g in _GENT)   # 256

GATE_M = np.array([
    [0, 0, 0, 0], [0, 0, 0, 1], [0, 1, 0, -1], [0, 1, 0, 0],
    [0, 0, 1, -1], [0, 0, 1, 0], [0, 1, 1, -2], [0, 1, 1, -1],
    [1, -1, -1, 1], [1, -1, -1, 2], [1, 0, -1, 0], [1, 0, -1, 1],
    [1, -1, 0, 0], [1, -1, 0, 1], [1, 0, 0, -1], [1, 0, 0, 0],
], dtype=np.float64)


def _softmax64(w):
    w = np.asarray(w, np.float64)
    e = np.exp(w - w.max(-1, keepdims=True))
    return e / e.sum(-1, keepdims=True)


# ---------------------------------------------------------------------------
# static op schedule with greedy engine assignment
# ---------------------------------------------------------------------------
def _schedule():
    """Software-pipelined op list.  Engines: 'v' DVE, 'a' Act; Pool only
    dispatches gathers (compute on Pool head-blocks the gather queue).
    Each L0 tile's combine ops (tt0/fix) are emitted one tile AFTER its
    side-ts ops, and merge ops one slot after the fix that enables them,
    so no op waits at an engine's queue head for a result another engine
    produced a moment ago.  The post-loop drain (final merge chain) is
    forced onto DVE -- it is latency-critical, and DVE has the cheapest
    per-op cost."""
    ops = []
    col = [0]
    loads = {'v': 0.0, 'a': 0.0}
    TSW = {'v': 1.38, 'a': 3.38}   # whole-tile tensor_scalar, measured us
    TTW = {'v': 2.15}              # whole-tile tensor_tensor
    ngather = [0]

    def pick(cost):
        e = min(cost, key=lambda k: loads[k] + cost[k])
        loads[e] += cost[e]
        return e

    def need_gather(upto):
        while ngather[0] <= min(upto, len(_GENT) - 1):
            ops.append({'kind': 'gather', 'g': ngather[0]})
            ngather[0] += 1

    def alloc_col():
        c = col[0]
        col[0] += 2
        return c

    fixed = set()
    mergeq = []

    def note_fixed(l, key, drain=False):
        fixed.add((l, key))
        if l < 4 and (l, key ^ 1) in fixed:
            mergeq.append((l + 1, key // 2))

    def emit_merge(drain, pool=False):
        l, key = mergeq.pop(0)
        c = alloc_col()
        e_tt = 'v' if drain else ('p' if pool else pick(TTW))
        e_ts = 'v' if drain else ('p' if pool else pick(TSW))
        ops.append({'kind': 'tt', 'l': l, 'key': key, 'eng': e_tt})
        ops.append({'kind': 'fix', 'l': l, 'key': key, 'col': c,
                    'eng': e_ts})
        note_fixed(l, key)

    pending = []
    for t in range(NT0):
        need_gather(_ENT_OF_SIDE[2 * t + 1][0] + 1)
        tail = t == NT0 - 1
        ops.append({'kind': 'ts_side', 'side': 0, 't': t,
                    'col': alloc_col(), 'eng': 'v' if tail else pick(TSW)})
        ops.append({'kind': 'ts_side', 'side': 1, 't': t,
                    'col': alloc_col(), 'eng': 'a' if tail else pick(TSW)})
        ops.extend(pending)
        if pending:
            note_fixed(0, t - 1)
        pending = [{'kind': 'tt0', 't': t,
                    'eng': 'v' if tail else pick(TTW)},
                   {'kind': 'fix', 'l': 0, 'key': t, 'col': alloc_col(),
                    'eng': 'v' if tail else pick(TSW)}]
        if mergeq:
            emit_merge(False)
        if mergeq:
            emit_merge(False)
    ops.extend(pending)
    note_fixed(0, NT0 - 1)
    while mergeq:
        emit_merge(True)
    # L5: node n merges groups (2n, 2n+1) of T4; realign to base-0 first.
    for n in (0, 1):
        c = alloc_col()
        ops.append({'kind': 'l5re', 'n': n})
        ops.append({'kind': 'tt5', 'n': n, 'eng': 'v'})
        ops.append({'kind': 'fix5', 'n': n, 'col': c, 'eng': 'v'})
    c = alloc_col()
    ops.append({'kind': 'tt6', 'eng': 'v'})
    for h in (0, 1):
        ops.append({'kind': 'fin', 'col': c, 'h': h})
    return ops, col[0]


_OPS, _NCOL = _schedule()

_LANES = np.arange(128)
_G = _LANES >> 5          # group 0..3
_KL = _LANES & 31         # kernel-in-core 0..31


def _node_of(l, key):
    """Tree-node index per lane for a level-l tile (l <= 4)."""
    if l == 0:
        return key + NT0 * _G
    return (_G << (4 - l)) + key


# ---------------------------------------------------------------------------
# host tables
# ---------------------------------------------------------------------------
def _build_tables(ws):
    """Per-(node,kern) scalars over the FULL K=64, f64."""
    cs = [np.einsum('skg,gj->skj', _softmax64(w), GATE_M) for w in ws]
    s_req = [np.ones((2 ** (DEPTH - l), K)) for l in range(DEPTH + 1)]
    t_req = [np.zeros((2 ** (DEPTH - l), K)) for l in range(DEPTH + 1)]
    for l in range(DEPTH, 0, -1):
        c = cs[l]
        c1, c2, c3 = c[..., 1], c[..., 2], c[..., 3]
        tA, tB = -c2 / c3, -c1 / c3
        t_req[l - 1][0::2], t_req[l - 1][1::2] = tA, tB
        s_req[l - 1][0::2] = (1 + np.abs(tA)) / 2
        s_req[l - 1][1::2] = (1 + np.abs(tB)) / 2
    c = cs[0]
    c1, c2, c3 = c[..., 1], c[..., 2], c[..., 3]
    ta, tb = -c2 / c3, -c1 / c3
    sa, sb = (1 + np.abs(ta)) / 2, (1 + np.abs(tb)) / 2
    l0ab = (1 / sa, -ta / sa, 1 / sb, -tb / sb)
    p2 = []
    for l in range(DEPTH + 1):
        c = cs[l]
        c0, c1, c2, c3 = c[..., 0], c[..., 1], c[..., 2], c[..., 3]
        if l == 0:
            tAc, tBc, sA, sB = ta, tb, sa, sb
        else:
            tAc, tBc = t_req[l - 1][0::2], t_req[l - 1][1::2]
            sA, sB = s_req[l - 1][0::2], s_req[l - 1][1::2]
        D0 = c0 + c1 * tAc + c2 * tBc + c3 * tAc * tBc
        p2.append((c3 * sA * sB / s_req[l], (D0 - t_req[l]) / s_req[l]))
    return l0ab, p2


def _coef_table(ws, kg):
    """[128, _NCOL] f32 for kernel-group kg (kernels 32kg..32kg+31)."""
    l0ab, p2 = _build_tables(ws)
    qa1, qa2, qb1, qb2 = l0ab
    kern = KPC * kg + _KL
    coef = np.zeros((128, _NCOL), dtype=np.float64)
    for op in _OPS:
        k = op['kind']
        if k == 'ts_side':
            s = _node_of(0, op['t'])
            q1, q2 = (qa1, qa2) if op['side'] == 0 else (qb1, qb2)
            coef[:, op['col']] = q1[s, kern]
            coef[:, op['col'] + 1] = q2[s, kern]
        elif k == 'fix':
            n = _node_of(op['l'], op['key'])
            al, be = p2[op['l']]
            coef[:, op['col']] = al[n, kern]
            coef[:, op['col'] + 1] = be[n, kern]
        elif k == 'fix5':
            al, be = p2[5]
            coef[0:32, op['col']] = al[op['n'], kern[0:32]]
            coef[0:32, op['col'] + 1] = be[op['n'], kern[0:32]]
        elif k == 'fin':
            al, be = p2[6]
            coef[0:32, op['col']] = al[0, kern[0:32]]
            coef[0:32, op['col'] + 1] = be[0, kern[0:32]]
    return coef.astype(np.float32)


def _gidx_table(idx_a, idx_b, kg):
    """int16 gather-row indices [128, GCOLS] for kernel-group kg.
    Entry e fetches sides _GENT[e]; row i = j*128 + p lands at
    table[i%16, _ENT_COL[e] + i//16]."""
    gidx = np.zeros((128, GCOLS), dtype=np.int64)
    kern = KPC * kg + _KL
    for e, sides in enumerate(_GENT):
        for j, sd in enumerate(sides):
            t, side = sd // 2, sd % 2
            idx = idx_a if side == 0 else idx_b
            s = _node_of(0, t)
            ha = idx[kern, 0, s, 0].astype(np.int64)
            wa = idx[kern, 0, s, 1].astype(np.int64)
            ca = idx[kern, 0, s, 2].astype(np.int64)
            val = (ha * 3 + wa) * C + ca
            i = j * 128 + _LANES
            gidx[i % 16, _ENT_COL[e] + i // 16] = val
    assert gidx.max() < ROWS
    return gidx.astype(np.int16)


def _xsh_core(x, bg):
    """[ROWS, RSTEP] fp16 for batch-group bg: row d*64+c = the compact
    30x30x4b crop of channel c at shift d=(dh*3+dw), zero-padded to FW."""
    xs = x[B4 * bg:B4 * bg + B4].transpose(1, 2, 3, 0)  # [C,H,W,B4]
    xsh = np.zeros((ROWS, RSTEP), dtype=np.float16)
    for dh in range(3):
        for dw in range(3):
            d = dh * 3 + dw
            for c in range(C):
                xsh[d * C + c, :FDAT] = \
                    xs[c, dh:dh + 30, dw:dw + 30, :].reshape(-1)
    return xsh


# ---------------------------------------------------------------------------
# numpy emulator of the exact device schedule (validation aid)
# ---------------------------------------------------------------------------
def _emulate_core(xsh, gidx, coef):
    def f16(v):
        return v.astype(np.float16).astype(np.float32)
    F2 = FW // 2
    xr = xsh.astype(np.float32)
    ab = {}
    tiles = {}
    tmp = {}
    x5 = {0: np.zeros((32, FW), np.float32), 1: np.zeros((32, FW), np.float32)}
    w5 = {}
    w6 = np.zeros((32, FW), np.float32)
    out = np.zeros((KPC, P * B4), dtype=np.float32)

    def hs(h):
        return slice(F2 * h, F2 * (h + 1))
    for op in _OPS:
        k = op['kind']
        if k == 'gather':
            e = op['g']
            nc = 8 * len(_GENT[e])
            cols = gidx[:, _ENT_COL[e]:_ENT_COL[e] + nc]
            flat = cols[:16, :].T.reshape(-1)
            dst = np.empty((128, len(_GENT[e]), FW), np.float32)
            for i, idx in enumerate(flat):
                dst[i % 128, i // 128] = xr[idx, :FW]
            ab[e] = dst
        elif k == 'ts_side':
            t, side = op['t'], op['side']
            e, j = _ENT_OF_SIDE[2 * t + side]
            a = ab[e][:, j, :]
            c = op['col']
            tmp[(t, side)] = f16(f16(a) * coef[:, c, None]
                                 + coef[:, c + 1, None])
        elif k == 'tt0':
            t = op['t']
            tmp[(t, 0)] = f16(tmp[(t, 0)] * tmp[(t, 1)])
        elif k == 'fix' and op['l'] == 0:
            c = op['col']
            tiles[(0, op['key'])] = f16(
                tmp[(op['key'], 0)] * coef[:, c, None] + coef[:, c + 1, None])
        elif k == 'tt':
            l, key = op['l'], op['key']
            tmp[('w', l, key)] = f16(tiles[(l - 1, 2 * key)] *
                                     tiles[(l - 1, 2 * key + 1)])
        elif k == 'fix':
            l, key, c = op['l'], op['key'], op['col']
            tiles[(l, key)] = f16(
                tmp[('w', l, key)] * coef[:, c, None] + coef[:, c + 1, None])
        elif k == 'l5re':
            pass
        elif k == 'tt5':
            n = op['n']
            T4 = tiles[(4, 0)]
            w5[n] = f16(T4[64 * n:64 * n + 32] * T4[64 * n + 32:64 * n + 64])
        elif k == 'fix5':
            n, c = op['n'], op['col']
            x5[n] = f16(
                w5[n] * coef[0:32, c, None] + coef[0:32, c + 1, None])
        elif k == 'tt6':
            w6 = f16(x5[0] * x5[1])
        elif k == 'fin':
            c, h = op['col'], op['h']
            half = P * B4 // 2
            o = (w6[:, half * h:half * (h + 1)] * coef[0:32, c, None]
                 + coef[0:32, c + 1, None])
            out[:, half * h:half * (h + 1)] = o
    return out


# ---------------------------------------------------------------------------
# Bass program
# ---------------------------------------------------------------------------
_BASS_CACHE = {}


def _build_bass(debug=False):
    ck = ('nc', debug)
    if ck in _BASS_CACHE:
        return _BASS_CACHE[ck]
    import concourse.bass as bass  # noqa: F401
    import concourse.mybir as mybir
    import concourse.tile as tile
    import concourse.bacc as bacc

    f32 = mybir.dt.float32
    f16 = mybir.dt.float16
    i16 = mybir.dt.int16
    AL = mybir.AluOpType
    ACTF = mybir.ActivationFunctionType

    nc = bacc.Bacc("TRN2", target_bir_lowering=False, debug=debug,
                   num_devices=NCORES, num_swdge_queues=2)
    xsh_d = nc.dram_tensor("xsh", [ROWS, RSTEP], f16, kind="ExternalInput").ap()
    gidx_d = nc.dram_tensor("gidx", [128, GCOLS], i16,
                            kind="ExternalInput").ap()
    coef_d = nc.dram_tensor("coef", [128, _NCOL], f32,
                            kind="ExternalInput").ap()
    out_d = nc.dram_tensor("out", [KPC, P * B4], f32,
                           kind="ExternalOutput").ap()

    with tile.TileContext(nc) as tc:
        with (
            tc.tile_pool(name="const", bufs=1) as pc,
            tc.tile_pool(name="gath", bufs=2) as pg,
            tc.tile_pool(name="tmp", bufs=3) as ptmp,
            tc.tile_pool(name="t0", bufs=2) as pt0,
            tc.tile_pool(name="lvl", bufs=2) as plv,
            tc.tile_pool(name="fin", bufs=1) as pfin,
            tc.tile_pool(name="outp", bufs=1) as pout,
        ):
            gidx_t = pc.tile([128, GCOLS], i16, tag="gidx",
                             name="gidx_t")
            nc.sync.dma_start(gidx_t[:], gidx_d[:])
            coef_t = pc.tile([128, _NCOL], f32, tag="coef", name="coef_t")
            nc.sync.dma_start(coef_t[:], coef_d[:])
            warm_t = pc.tile([1, 8], f32, tag="warm", name="warm_t")
            nc.scalar.activation(warm_t[:], coef_t[0:1, 0:8],
                                 ACTF.Identity, bias=0.0, scale=1.0)

            eng = {'v': nc.vector, 'a': nc.scalar, 'p': nc.gpsimd}

            def ts(e, out_ap, in_ap, col, rows=slice(0, 128)):
                s1 = coef_t[rows, col:col + 1]
                s2 = coef_t[rows, col + 1:col + 2]
                if e == 'a':
                    nc.scalar.activation(out_ap, in_ap, ACTF.Identity,
                                         bias=s2, scale=s1)
                else:
                    eng[e].tensor_scalar(out=out_ap, in0=in_ap, scalar1=s1,
                                         scalar2=s2, op0=AL.mult, op1=AL.add)

            ab = {}
            tmp = {}
            tiles = {}
            x5 = {}
            ra5 = {}
            F2 = FW // 2
            xsh_view = xsh_d[:, 0:FW]

            def hsl(h):
                return slice(F2 * h, F2 * (h + 1))
            for op in _OPS:
                k = op['kind']
                if k == 'gather':
                    e = op['g']
                    ns = len(_GENT[e])
                    t_ab = pg.tile([128, 2 * FW], f16, tag="AB",
                                   name="ab")
                    ab[e] = t_ab
                    nc.gpsimd.dma_gather(
                        out_ap=t_ab[:, 0:ns * FW].rearrange(
                            "p (j e) -> p j e", j=ns, e=FW),
                        in_ap=xsh_view,
                        idxs_ap=gidx_t[:, _ENT_COL[e]:
                                       _ENT_COL[e] + 8 * ns],
                        num_idxs=128 * ns,
                        num_idxs_reg=128 * ns,
                        elem_size=FW,
                        elem_step=RSTEP,
                    )
                elif k == 'ts_side':
                    t, side = op['t'], op['side']
                    e, j = _ENT_OF_SIDE[2 * t + side]
                    src = ab[e][:, j * FW:(j + 1) * FW]
                    dst = ptmp.tile([128, FW], f16, tag="ab"[side],
                                    name="ab"[side])
                    tmp[(t, side)] = dst
                    ts(op['eng'], dst[:], src, op['col'])
                elif k == 'tt0':
                    t = op['t']
                    eng[op['eng']].tensor_tensor(
                        tmp[(t, 0)][:], tmp[(t, 1)][:], tmp[(t, 0)][:],
                        AL.mult)
                elif k == 'fix' and op['l'] == 0:
                    dst = pt0.tile([128, FW], f16, tag="T0",
                                   name=f"t0_{op['key']}")
                    tiles[(0, op['key'])] = dst
                    ts(op['eng'], dst[:], tmp[(op['key'], 0)][:], op['col'])
                elif k == 'tt':
                    l, key = op['l'], op['key']
                    tA = tiles[(l - 1, 2 * key)]
                    tB = tiles[(l - 1, 2 * key + 1)]
                    eng[op['eng']].tensor_tensor(
                        tA[:], tB[:], tA[:], AL.mult)
                elif k == 'fix':
                    l, key = op['l'], op['key']
                    pool = pfin if l == 4 else plv
                    tiles[(l, key)] = pool.tile(
                        [128, FW], f16, tag=f"T{l}", name=f"t{l}_{key}")
                    src = tiles[(l - 1, 2 * key)]
                    ts(op['eng'], tiles[(l, key)][:], src[:], op['col'])
                elif k == 'l5re':
                    # merges cross lane groups: copy T4 slabs to base 0
                    n = op['n']
                    T4 = tiles[(4, 0)]
                    ra = pfin.tile([32, FW], f16, tag=f"r{n}", name=f"r{n}")
                    nc.sync.dma_start(ra[:], T4[64 * n + 32:64 * n + 64, :])
                    if n == 0:
                        left = T4[0:32, :]
                    else:
                        rl = pfin.tile([32, FW], f16, tag="rl", name="rl")
                        nc.sync.dma_start(rl[:], T4[64:96, :])
                        left = rl[:]
                    ra5[n] = (left, ra)
                    if n == 1:
                        x5[1] = pfin.tile([32, FW], f16, tag="X51",
                                          name="x51")
                elif k == 'tt5':
                    n = op['n']
                    left, ra = ra5[n]
                    eng[op['eng']].tensor_tensor(
                        ra[:], left, ra[:], AL.mult)
                elif k == 'fix5':
                    n, c = op['n'], op['col']
                    if n == 0:
                        # write X5(0) straight over r0's buffer
                        x5[0] = ra5[0][1]
                        ts(op['eng'], x5[0][:], ra5[0][1][:],
                           c, rows=slice(0, 32))
                    else:
                        ts(op['eng'], x5[1][:], ra5[1][1][:],
                           c, rows=slice(0, 32))
                elif k == 'tt6':
                    eng[op['eng']].tensor_tensor(
                        x5[0][:], x5[1][:], x5[0][:], AL.mult)
                elif k == 'fin':
                    # compact layout: output halves are contiguous slices
                    c, h = op['col'], op['h']
                    half = P * B4 // 2
                    w6v = x5[0][:, half * h:half * (h + 1)]
                    out_t = pout.tile([KPC, half], f32, tag=f"out{h}",
                                      name=f"outt{h}")
                    nc.scalar.activation(
                        out_t[:], w6v, ACTF.Identity,
                        bias=coef_t[0:KPC, c + 1:c + 2],
                        scale=coef_t[0:KPC, c:c + 1])
                    nc.sync.dma_start(
                        out_d[:, half * h:half * (h + 1)], out_t[:])
    nc.compile()
    _BASS_CACHE[ck] = nc
    return nc


# ---------------------------------------------------------------------------
# entry points
# ---------------------------------------------------------------------------
def _prep_inputs(x, idx_a, idx_b, ws):
    x = np.ascontiguousarray(x, dtype=np.float32)
    coefs = [_coef_table(ws, kg) for kg in range(2)]
    gidxs = [_gidx_table(idx_a, idx_b, kg) for kg in range(2)]
    xshs = [_xsh_core(x, bg) for bg in range(4)]
    in_maps = []
    for core in range(NCORES):
        bg, kg = core % 4, core // 4
        in_maps.append({"xsh": xshs[bg], "gidx": gidxs[kg],
                        "coef": coefs[kg]})
    return in_maps


def _assemble(core_outs):
    full = np.zeros((B, K, P, 1), dtype=np.float32)
    for core, o in enumerate(core_outs):
        bg, kg = core % 4, core // 4
        o = np.asarray(o, np.float32).reshape(KPC, P, B4)
        full[B4 * bg:B4 * bg + B4, KPC * kg:KPC * kg + KPC, :, 0] = \
            o.transpose(2, 0, 1)
    return full


def kernel(x, idx_a, idx_b, w0, w1, w2, w3, w4, w5, w6):
    ws = [np.asarray(w, dtype=np.float32) for w in
          (w0, w1, w2, w3, w4, w5, w6)]
    x = np.asarray(x, dtype=np.float32)
    idx_a = np.asarray(idx_a, dtype=np.int32)
    idx_b = np.asarray(idx_b, dtype=np.int32)
    in_maps = _prep_inputs(x, idx_a, idx_b, ws)
    nc = _build_bass()
    from concourse.bass_utils import run_bass_kernel_spmd
    res = run_bass_kernel_spmd(nc, in_maps, core_ids=list(range(NCORES)))
    return _assemble([r["out"] for r in res.results])


def kernel_emulate(x, idx_a, idx_b, w0, w1, w2, w3, w4, w5, w6):
    """Pure-numpy emulation of the exact device schedule (debug aid)."""
    ws = [np.asarray(w, dtype=np.float32) for w in
          (w0, w1, w2, w3, w4, w5, w6)]
    in_maps = _prep_inputs(np.asarray(x, np.float32),
                           np.asarray(idx_a, np.int32),
                           np.asarray(idx_b, np.int32), ws)
    outs = [_emulate_core(m["xsh"], m["gidx"].astype(np.int64), m["coef"])
            for m in in_maps]
    return _assemble(outs)
